# revision 14
# baseline (speedup 1.0000x reference)
"""Bass/Tile TRN2 kernel for nn_BigramLanguageModel (8-layer dense transformer).

Strategy: pure data-parallel over batch across the 8 NeuronCores (8 batch
items / core, no collectives). On-device, the residual stream is kept
feature-major ([C, tokens]) in SBUF so every matmul contracts over the
partition dim without any transposes:

  - LN stats (sum, sum-of-squares over C) via ones-column matmuls on PE,
    finished on DVE/ACT, applied with partition-broadcast DMA'd (A, B) rows:
    h = x*A + B   (A = rstd, B = -mean*rstd; gamma/beta are folded into the
    following weight matrices on the host, which is exact).
  - QKV / proj / FFN as accumulating PSUM matmuls; biases folded/fused into
    the PSUM->SBUF evacuation ops.
  - Attention per (batch-item, head): scores computed [s, t] so the masked
    exp needs no max-subtraction (scores are tiny by construction), the
    denominator comes for free from a ones-column appended to V, and the
    1/den normalization is a partition-broadcast DMA + one DVE multiply.

Matmul operands are fp16 (fp32 PSUM accumulation); the residual stream, LN
stats and softmax normalization stay fp32.  The lm-head runs fp32 (float32r).
"""

import os
import sys
from contextlib import ExitStack

import numpy as np

for _p in ("/opt/trn_rl_repo", "/root/.axon_site/_ro/trn_rl_repo"):
    if os.path.isdir(_p) and _p not in sys.path:
        sys.path.insert(0, _p)
        break

import concourse.bass as bass
import concourse.mybir as mybir
import concourse.tile as tile
from concourse import bacc

# model config (hardcoded per problem spec)
B, T, C, H, L, V = 64, 256, 512, 8, 8, 100
HD = C // H          # 64
FF = 4 * C           # 2048
EPS = 1e-5
NCORES = 8
BL = B // NCORES     # 8 batch items per core
NT = BL * T          # 2048 tokens per core
P = 128
NCC = C // P         # 4 c-chunks
NFF = FF // P        # 16 ff-chunks
TCH = 512            # token chunk (2 batch items)
NTC = NT // TCH      # 4
BI = TCH // T        # 2 batch items per token chunk

F32 = mybir.dt.float32
F16 = mybir.dt.float16
F32R = mybir.dt.float32r
ADD = mybir.AluOpType.add
MULT = mybir.AluOpType.mult
AF = mybir.ActivationFunctionType


def _r(ap):
    """view an fp32 AP as float32r for full-rate full-precision matmul"""
    return ap if ap.dtype == F32R else ap.bitcast(F32R)


def _bcast(row_ap, parts):
    """broadcast a [1, N]-ish AP along the partition dim (for DMA use)"""
    return bass.AP(
        tensor=row_ap.tensor,
        offset=row_ap.offset,
        ap=[[0, parts]] + [list(d) for d in row_ap.ap[1:]],
    )


def _bcast_dram(vec_ap, parts):
    """broadcast a 1-D DRAM AP along a new leading (partition) dim"""
    return bass.AP(
        tensor=vec_ap.tensor,
        offset=vec_ap.offset,
        ap=[[0, parts]] + [list(d) for d in vec_ap.ap],
    )


def build_bass():
    nc = bacc.Bacc()
    dp = nc.declare_dram_parameter

    onehot_d = dp("onehotT", [V, NT], F16, False)     # per-core (from idx)
    tok_d = dp("tok_emb16", [V, C], F16, False)
    pos2_d = dp("pos2T", [C, TCH], F32, False)        # pos_emb.T tiled x2
    mask_d = dp("maskT", [P, 2, T], F16, False)       # causal mask [s%128, s//128, t]
    wq_d = dp("wq", [L, C, C], F16, False)            # folded: ln1_g * Wq / sqrt(C)
    wk_d = dp("wk", [L, C, C], F16, False)
    wv_d = dp("wv", [L, C, C], F16, False)
    wo_d = dp("wo", [L, C, C], F16, False)
    w1_d = dp("w1", [L, C, FF], F16, False)           # folded: ln2_g * W1
    w2_d = dp("w2", [L, FF, C], F16, False)
    bq_d = dp("bq", [L, C], F32, False)
    bk_d = dp("bk", [L, C], F32, False)
    bv_d = dp("bv", [L, C], F32, False)
    bo_d = dp("bo", [L, C], F32, False)
    b1_d = dp("b1", [L, FF], F32, False)
    b2_d = dp("b2", [L, C], F32, False)
    wlm_d = dp("wlm", [C, V], F32, False)
    blm_d = dp("blm", [V], F32, False)
    out_d = dp("out", [NT, V], F32, True)

    with tile.TileContext(nc) as tc, ExitStack() as ctx:
        # ---------------- pools ----------------
        pconst = ctx.enter_context(tc.tile_pool(name="const", bufs=1))
        px = ctx.enter_context(tc.tile_pool(name="x", bufs=1))
        pw = ctx.enter_context(tc.tile_pool(name="w", bufs=1))
        pbias = ctx.enter_context(tc.tile_pool(name="bias", bufs=1))
        ph = ctx.enter_context(tc.tile_pool(name="h", bufs=2))
        pq = ctx.enter_context(tc.tile_pool(name="q", bufs=1))
        pv = ctx.enter_context(tc.tile_pool(name="v", bufs=2))
        po = ctx.enter_context(tc.tile_pool(name="o", bufs=1))
        pffn = ctx.enter_context(tc.tile_pool(name="ffn", bufs=1))
        psq = ctx.enter_context(tc.tile_pool(name="sq", bufs=2))
        pstat = ctx.enter_context(tc.tile_pool(name="stat", bufs=2))
        pe_ = ctx.enter_context(tc.tile_pool(name="e", bufs=4))
        prd = ctx.enter_context(tc.tile_pool(name="rd", bufs=4))
        plog = ctx.enter_context(tc.tile_pool(name="log", bufs=2))
        # PSUM pools (8 banks total: 3 + 2 + 2 + 1)
        pmm = ctx.enter_context(tc.tile_pool(name="mm", bufs=3, space="PSUM"))
        psc = ctx.enter_context(tc.tile_pool(name="scps", bufs=2, space="PSUM"))
        pops = ctx.enter_context(tc.tile_pool(name="ops", bufs=1, space="PSUM"))
        pst = ctx.enter_context(tc.tile_pool(name="stps", bufs=1, space="PSUM"))

        # ---------------- constants ----------------
        ones_f = pconst.tile([P, 1], F32, tag="ones_f", name="ones_f")
        nc.vector.memset(ones_f, 1.0)
        ones = pconst.tile([P, 1], F32R, tag="ones", name="ones")
        nc.vector.tensor_copy(ones, ones_f)
        ones1_f = pconst.tile([1, P], F32, tag="ones1_f", name="ones1_f")
        nc.vector.memset(ones1_f, 1.0)
        ones1 = pconst.tile([1, P], F32R, tag="ones1", name="ones1")
        nc.vector.tensor_copy(ones1, ones1_f)
        eps_t = pconst.tile([1, 1], F32, tag="eps", name="eps")
        nc.vector.memset(eps_t, EPS)
        mask_sb = pconst.tile([P, 2, T], F16, tag="mask", name="mask")
        nc.sync.dma_start(out=mask_sb, in_=mask_d[:, :, :])
        tok_sb = pconst.tile([V, C], F16, tag="tok", name="tok")
        nc.sync.dma_start(out=tok_sb, in_=tok_d[:, :])
        wlm_sb = []
        for cc in range(NCC):
            t = pconst.tile([P, V], F32, tag=f"wlm{cc}", name=f"wlm{cc}")
            nc.sync.dma_start(out=t, in_=wlm_d[cc * P:(cc + 1) * P, :])
            wlm_sb.append(t)
        blm_bc = pconst.tile([P, V], F32, tag="blm", name="blm")
        nc.sync.dma_start(out=blm_bc, in_=_bcast_dram(blm_d[:], P))

        # resident residual stream, feature-major: x_T[c, t]
        x_sb = [px.tile([P, NT], F32R, tag=f"x{cc}", name=f"x{cc}") for cc in range(NCC)]

        # ---------------- embedding ----------------
        with tc.tile_pool(name="emb", bufs=1) as pemb:
            oh_sb = pemb.tile([V, NT], F16, tag="oh", name="oh")
            nc.sync.dma_start(out=oh_sb, in_=onehot_d[:, :])
            pos_sb = []
            for cc in range(NCC):
                t = pemb.tile([P, TCH], F32, tag=f"pos{cc}", name=f"pos{cc}")
                nc.sync.dma_start(out=t, in_=pos2_d[cc * P:(cc + 1) * P, :])
                pos_sb.append(t)
            for ti in range(NTC):
                tsl = slice(ti * TCH, (ti + 1) * TCH)
                for cc in range(NCC):
                    ps = pmm.tile([P, TCH], F32, tag="mm", name="mmps")
                    nc.tensor.matmul(ps, tok_sb[:, cc * P:(cc + 1) * P],
                                     oh_sb[:, tsl], start=True, stop=True)
                    nc.vector.tensor_add(x_sb[cc][:, tsl], ps, pos_sb[cc])

        # ---------------- helpers ----------------
        def emit_ln(src, tsl, htag):
            """emit LN over feature dim for token slice tsl; returns fp16 tiles"""
            S0 = pst.tile([1, TCH], F32, tag="sum", name="S0")
            S1 = pst.tile([1, TCH], F32, tag="ssq", name="S1")
            for cc in range(NCC):
                sq = psq.tile([P, TCH], F32R, tag="sq", name="sq")
                nc.vector.tensor_mul(sq, src[cc][:, tsl], src[cc][:, tsl])
                nc.tensor.matmul(S0[0:1, :], _r(ones[:, :]), _r(src[cc][:, tsl]),
                                 start=(cc == 0), stop=(cc == NCC - 1))
                nc.tensor.matmul(S1[0:1, :], _r(ones[:, :]), _r(sq[:, :]),
                                 start=(cc == 0), stop=(cc == NCC - 1))
            m_t = pstat.tile([1, TCH], F32R, tag="m", name="m_t")
            v_t = pstat.tile([1, TCH], F32R, tag="v", name="v_t")
            m2_t = pstat.tile([1, TCH], F32, tag="m2", name="m2_t")
            nc.vector.tensor_scalar_mul(m_t, S0[0:1, :], 1.0 / C)
            nc.vector.tensor_scalar_mul(v_t, S1[0:1, :], 1.0 / C)
            nc.vector.tensor_mul(m2_t, m_t, m_t)
            nc.vector.tensor_sub(v_t, v_t, m2_t)          # var = E[x^2] - m^2
            nc.scalar.activation(v_t, v_t, AF.Sqrt, bias=eps_t[:, :], scale=1.0)
            with nc.allow_low_precision("fp32r rstd is fp32-equivalent here"):
                nc.vector.reciprocal(v_t, v_t)            # rstd
            nc.vector.scalar_tensor_tensor(m_t, m_t, -1.0, v_t,
                                           op0=MULT, op1=MULT)  # -m*rstd
            a_ps = pmm.tile([P, TCH], F32, tag="mm", name="a_ps")
            nc.tensor.matmul(a_ps, _r(ones1[:, :]), _r(v_t[:, :]),
                             start=True, stop=True)
            b_ps = pmm.tile([P, TCH], F32, tag="mm", name="b_ps")
            nc.tensor.matmul(b_ps, _r(ones1[:, :]), _r(m_t[:, :]),
                             start=True, stop=True)
            h = []
            for cc in range(NCC):
                d = ph.tile([P, TCH], F16, tag=f"{htag}{cc}", name=f"h{cc}")
                nc.vector.tensor_mul(d, src[cc][:, tsl], a_ps)
                nc.vector.tensor_add(d, d, b_ps)
                h.append(d)
            return h

        # ---------------- transformer layers ----------------
        for l in range(L):
            # layer weights (fp16) + biases
            def _load_w(dram, tag, n, width):
                ts_ = []
                for i in range(n):
                    t = pw.tile([P, width], F16, tag=f"{tag}{i}", name=f"{tag}{i}")
                    nc.sync.dma_start(out=t, in_=dram[l, i * P:(i + 1) * P, :])
                    ts_.append(t)
                return ts_

            wq_sb = _load_w(wq_d, "wq", NCC, C)
            wk_sb = _load_w(wk_d, "wk", NCC, C)
            wv_sb = _load_w(wv_d, "wv", NCC, C)
            wo_sb = _load_w(wo_d, "wo", NCC, C)
            w1_sb = _load_w(w1_d, "w1", NCC, FF)
            w2_sb = _load_w(w2_d, "w2", NFF, C)
            bq_sb = pbias.tile([P, NCC], F32, tag="bq", name="bq")
            nc.sync.dma_start(out=bq_sb, in_=bq_d[l].rearrange("(a p) -> p a", p=P))
            bk_sb = pbias.tile([P, NCC], F32, tag="bk", name="bk")
            nc.sync.dma_start(out=bk_sb, in_=bk_d[l].rearrange("(a p) -> p a", p=P))
            bo_sb = pbias.tile([P, NCC], F32, tag="bo", name="bo")
            nc.sync.dma_start(out=bo_sb, in_=bo_d[l].rearrange("(a p) -> p a", p=P))
            b2_sb = pbias.tile([P, NCC], F32, tag="b2", name="b2")
            nc.sync.dma_start(out=b2_sb, in_=b2_d[l].rearrange("(a p) -> p a", p=P))
            b1_sb = pbias.tile([P, NFF], F32, tag="b1", name="b1")
            nc.sync.dma_start(out=b1_sb, in_=b1_d[l].rearrange("(a p) -> p a", p=P))
            bv_bc = pbias.tile([P, C], F32, tag="bvb", name="bvb")
            nc.sync.dma_start(out=bv_bc, in_=_bcast_dram(bv_d[l], P))
            bv_v = bv_bc[:].rearrange("p (h d) -> p h d", h=H)

            for ti in range(NTC):
                tsl = slice(ti * TCH, (ti + 1) * TCH)
                # ---- LN1 -> h1 (fp16, feature-major) ----
                h1 = emit_ln(x_sb, tsl, "h")

                # ---- QKV ----
                q_t, k_t = [], []
                for hq in range(NCC):
                    ps = pmm.tile([P, TCH], F32, tag="mm", name="mmps")
                    for cc in range(NCC):
                        nc.tensor.matmul(ps, wq_sb[cc][:, hq * P:(hq + 1) * P],
                                         h1[cc][:, :], start=(cc == 0),
                                         stop=(cc == NCC - 1))
                    qt = pq.tile([P, TCH], F16, tag=f"q{hq}", name=f"qt{hq}")
                    nc.scalar.activation(qt, ps, AF.Identity,
                                         bias=bq_sb[:, hq:hq + 1], scale=1.0)
                    q_t.append(qt)
                for hq in range(NCC):
                    ps = pmm.tile([P, TCH], F32, tag="mm", name="mmps")
                    for cc in range(NCC):
                        nc.tensor.matmul(ps, wk_sb[cc][:, hq * P:(hq + 1) * P],
                                         h1[cc][:, :], start=(cc == 0),
                                         stop=(cc == NCC - 1))
                    kt = pq.tile([P, TCH], F16, tag=f"k{hq}", name=f"kt{hq}")
                    nc.scalar.activation(kt, ps, AF.Identity,
                                         bias=bk_sb[:, hq:hq + 1], scale=1.0)
                    k_t.append(kt)
                v8 = []
                for tt in range(TCH // P):
                    ps = pmm.tile([P, C], F32, tag="mm", name="mmps")
                    for cc in range(NCC):
                        nc.tensor.matmul(ps, h1[cc][:, tt * P:(tt + 1) * P],
                                         wv_sb[cc][:, :], start=(cc == 0),
                                         stop=(cc == NCC - 1))
                    vt = pv.tile([P, H, HD + 1], F16, tag=f"v{tt}", name=f"vt{tt}")
                    nc.vector.tensor_add(vt[:, :, 0:HD],
                                         ps[:].rearrange("p (h d) -> p h d", h=H),
                                         bv_v)
                    nc.vector.memset(vt[:, :, HD:HD + 1], 1.0)
                    v8.append(vt)

                # ---- attention (per batch-item, per head) ----
                o_t = [po.tile([P, TCH], F16, tag=f"o{hq}", name=f"ot{hq}") for hq in range(NCC)]
                for bi in range(BI):
                    for hh in range(H):
                        hq, hr = divmod(hh, 2)
                        rsl = slice(hr * HD, (hr + 1) * HD)
                        qsl = q_t[hq][rsl, bi * T:(bi + 1) * T]
                        sc_ps = psc.tile([P, 2, T], F32, tag="scps", name="scps")
                        for sc in range(2):
                            ksl = k_t[hq][rsl, bi * T + sc * P: bi * T + (sc + 1) * P]
                            nc.tensor.matmul(sc_ps[:, sc, :], ksl, qsl,
                                             start=True, stop=True)
                        e = pe_.tile([P, 2, T], F16, tag="e", name="e")
                        nc.scalar.activation(e, sc_ps, AF.Exp)
                        nc.vector.tensor_mul(e, e, mask_sb)
                        o_ps = pops.tile([HD + 1, T], F32, tag="ops", name="ops")
                        for sc in range(2):
                            nc.tensor.matmul(o_ps, v8[bi * 2 + sc][:, hh, :],
                                             e[:, sc, :], start=(sc == 0),
                                             stop=(sc == 1))
                        rd = prd.tile([1, T], F32R, tag="rd", name="rd")
                        with nc.allow_low_precision("fp32r rden is fp32-equivalent"):
                            nc.vector.reciprocal(rd, o_ps[HD:HD + 1, :])
                        rdb_ps = pmm.tile([HD, T], F32, tag="mm", name="rdb_ps")
                        nc.tensor.matmul(rdb_ps, _r(ones1[:, 0:HD]), _r(rd[:, :]),
                                         start=True, stop=True)
                        osl = o_t[hq][rsl, bi * T:(bi + 1) * T]
                        nc.scalar.copy(osl, o_ps[0:HD, :])
                        nc.vector.tensor_mul(osl, osl, rdb_ps)

                # ---- proj + residual ----
                for cc in range(NCC):
                    ps = pmm.tile([P, TCH], F32, tag="mm", name="mmps")
                    for hq in range(NCC):
                        nc.tensor.matmul(ps, wo_sb[hq][:, cc * P:(cc + 1) * P],
                                         o_t[hq][:, :], start=(hq == 0),
                                         stop=(hq == NCC - 1))
                    nc.vector.scalar_tensor_tensor(
                        x_sb[cc][:, tsl], ps, bo_sb[:, cc:cc + 1],
                        x_sb[cc][:, tsl], op0=ADD, op1=ADD)

                # ---- LN2 -> h2; FFN ----
                h2 = emit_ln(x_sb, tsl, "g")
                ffn1 = []
                for fc in range(NFF):
                    ps = pmm.tile([P, TCH], F32, tag="mm", name="mmps")
                    for cc in range(NCC):
                        nc.tensor.matmul(ps, w1_sb[cc][:, fc * P:(fc + 1) * P],
                                         h2[cc][:, :], start=(cc == 0),
                                         stop=(cc == NCC - 1))
                    ft = pffn.tile([P, TCH], F16, tag=f"f{fc}", name=f"ft{fc}")
                    nc.scalar.activation(ft, ps, AF.Relu,
                                         bias=b1_sb[:, fc:fc + 1], scale=1.0)
                    ffn1.append(ft)
                for cc in range(NCC):
                    ps = pmm.tile([P, TCH], F32, tag="mm", name="mmps")
                    for fc in range(NFF):
                        nc.tensor.matmul(ps, w2_sb[fc][:, cc * P:(cc + 1) * P],
                                         ffn1[fc][:, :], start=(fc == 0),
                                         stop=(fc == NFF - 1))
                    nc.vector.scalar_tensor_tensor(
                        x_sb[cc][:, tsl], ps, b2_sb[:, cc:cc + 1],
                        x_sb[cc][:, tsl], op0=ADD, op1=ADD)

        # ---------------- lm head ----------------
        for tt in range(NT // P):
            ps = pmm.tile([P, V], F32, tag="mm", name="mmps")
            for cc in range(NCC):
                nc.tensor.matmul(ps, x_sb[cc][:, tt * P:(tt + 1) * P].bitcast(F32),
                                 wlm_sb[cc][:, :], start=(cc == 0),
                                 stop=(cc == NCC - 1))
            lo = plog.tile([P, V], F32, tag="lg", name="lo")
            nc.vector.tensor_add(lo, ps, blm_bc)
            nc.sync.dma_start(out=out_d[tt * P:(tt + 1) * P, :], in_=lo)

    if not nc.is_finalized():
        nc.finalize()
    return nc


def prep_inputs(idx, tok_emb, pos_emb, Wq, Wk, Wv, Wo, bo, ln1_g, ln1_b,
                ln2_g, ln2_b, W1, b1, W2, b2, Wlm, blm):
    """host-side: fold LN affines into weights, build per-core input maps"""
    f32 = np.float32
    idx = np.asarray(idx)
    tok_emb = np.asarray(tok_emb, f32)
    pos_emb = np.asarray(pos_emb, f32)
    scale = C ** -0.5

    wq = np.empty((L, C, C), f32)
    wk = np.empty((L, C, C), f32)
    wv = np.empty((L, C, C), f32)
    wo = np.empty((L, C, C), f32)
    w1 = np.empty((L, C, FF), f32)
    w2 = np.empty((L, FF, C), f32)
    bq = np.empty((L, C), f32)
    bk = np.empty((L, C), f32)
    bv = np.empty((L, C), f32)
    b1f = np.empty((L, FF), f32)
    for l in range(L):
        wq_c = np.asarray(Wq[l], f32).transpose(1, 0, 2).reshape(C, C)
        wk_c = np.asarray(Wk[l], f32).transpose(1, 0, 2).reshape(C, C)
        wv_c = np.asarray(Wv[l], f32).transpose(1, 0, 2).reshape(C, C)
        g1 = np.asarray(ln1_g[l], f32)[:, None]
        b1_ = np.asarray(ln1_b[l], f32)
        g2 = np.asarray(ln2_g[l], f32)[:, None]
        b2_ = np.asarray(ln2_b[l], f32)
        wq[l] = g1 * wq_c * scale
        bq[l] = (b1_ @ wq_c) * scale
        wk[l] = g1 * wk_c
        bk[l] = b1_ @ wk_c
        wv[l] = g1 * wv_c
        bv[l] = b1_ @ wv_c
        wo[l] = np.asarray(Wo[l], f32)
        w1[l] = g2 * np.asarray(W1[l], f32)
        b1f[l] = np.asarray(b1[l], f32) + b2_ @ np.asarray(W1[l], f32)
        w2[l] = np.asarray(W2[l], f32)

    # causal mask in [s%128, s//128, t] layout
    s_g = np.arange(2 * P).reshape(2, P).T          # [128, 2] global s
    mask = (s_g[:, :, None] <= np.arange(T)[None, None, :]).astype(np.float16)

    pos2 = np.concatenate([pos_emb.T, pos_emb.T], axis=1)  # [C, 512]

    shared = {
        "tok_emb16": tok_emb.astype(np.float16),
        "pos2T": np.ascontiguousarray(pos2, f32),
        "maskT": np.ascontiguousarray(mask),
        "wq": wq.astype(np.float16), "wk": wk.astype(np.float16),
        "wv": wv.astype(np.float16), "wo": wo.astype(np.float16),
        "w1": w1.astype(np.float16), "w2": w2.astype(np.float16),
        "bq": bq, "bk": bk, "bv": bv,
        "bo": np.asarray(bo, f32), "b1": b1f, "b2": np.asarray(b2, f32),
        "wlm": np.asarray(Wlm, f32), "blm": np.asarray(blm, f32),
    }
    in_maps = []
    vocab = np.arange(V)
    for core in range(NCORES):
        toks = np.asarray(idx[core * BL:(core + 1) * BL]).reshape(-1)
        oh = (vocab[:, None] == toks[None, :]).astype(np.float16)
        m = dict(shared)
        m["onehotT"] = np.ascontiguousarray(oh)
        in_maps.append(m)
    return in_maps


_NC_CACHE = {}


def get_nc():
    if "nc" not in _NC_CACHE:
        _NC_CACHE["nc"] = build_bass()
    return _NC_CACHE["nc"]


def run(in_maps, trace=False, **kw):
    from concourse.bass_utils import run_bass_kernel_spmd
    nc = get_nc()
    return run_bass_kernel_spmd(nc, in_maps, list(range(NCORES)), trace=trace, **kw)


def kernel(**inputs):
    in_maps = prep_inputs(**inputs)
    res = run(in_maps)
    outs = [res.results[i]["out"].reshape(BL, T, V) for i in range(NCORES)]
    return np.concatenate(outs, axis=0).astype(np.float32)


# revision 19
# speedup vs baseline: 1.1520x; 1.1520x over previous
"""Bass/Tile TRN2 kernel for nn_BigramLanguageModel (8-layer dense transformer).

Strategy: pure data-parallel over batch across the 8 NeuronCores (8 batch
items / core, no collectives). On-device, the residual stream is kept
feature-major ([C, tokens]) in SBUF so every matmul contracts over the
partition dim without any transposes:

  - LN stats (sum, sum-of-squares over C) via ones-column fp32r matmuls on
    PE, finished on DVE, applied as h = x*A + B with (A, B) rows broadcast
    across partitions by tiny K=1 PE matmuls into PSUM (gamma/beta are folded
    into the following weight matrices on the host, which is exact).
  - QKV / proj / FFN as accumulating PSUM matmuls; biases fused into the
    PSUM->SBUF evacuation ops (and skipped entirely when they are zero).
  - Attention per (batch-item, head): scores computed [s, t] so the masked
    exp needs no max-subtraction (scores are tiny by construction), the
    denominator comes free from a ones-column appended to V, and the
    normalization is a K=1 PE broadcast + one DVE divide. The mask multiply
    runs on the otherwise-idle GPSIMD engine.
  - The LN1 chain of each token chunk is software-pipelined behind the
    previous chunk's FFN2 matmuls to keep PE fed.

Matmul operands are fp16 (fp32 PSUM accumulation); the residual stream, LN
stats and softmax normalization stay fp32(r). The lm-head runs plain fp32.
"""

import os
import sys
from contextlib import ExitStack

import numpy as np

for _p in ("/opt/trn_rl_repo", "/root/.axon_site/_ro/trn_rl_repo"):
    if os.path.isdir(_p) and _p not in sys.path:
        sys.path.insert(0, _p)
        break

import concourse.bass as bass
import concourse.mybir as mybir
import concourse.tile as tile
from concourse import bacc

# model config (hardcoded per problem spec)
B, T, C, H, L, V = 64, 256, 512, 8, 8, 100
HD = C // H          # 64
FF = 4 * C           # 2048
EPS = 1e-5
NCORES = 8
BL = B // NCORES     # 8 batch items per core
NT = BL * T          # 2048 tokens per core
P = 128
NCC = C // P         # 4 c-chunks
NFF = FF // P        # 16 ff-chunks
TCH = 512            # token chunk (2 batch items)
NTC = NT // TCH      # 4
BI = TCH // T        # 2 batch items per token chunk

F32 = mybir.dt.float32
F16 = mybir.dt.float16
F32R = mybir.dt.float32r
ADD = mybir.AluOpType.add
MULT = mybir.AluOpType.mult
AF = mybir.ActivationFunctionType


def _r(ap):
    """view an fp32 AP as float32r for full-rate full-precision matmul"""
    return ap if ap.dtype == F32R else ap.bitcast(F32R)


def _bcast_dram(vec_ap, parts):
    """broadcast a 1-D DRAM AP along a new leading (partition) dim"""
    return bass.AP(
        tensor=vec_ap.tensor,
        offset=vec_ap.offset,
        ap=[[0, parts]] + [list(d) for d in vec_ap.ap],
    )


def build_bass(zero_attn_bias=False, zero_mlp_bias=False):
    nc = bacc.Bacc()
    dp = nc.declare_dram_parameter

    onehot_d = dp("onehotT", [V, NT], F16, False)
    tok_d = dp("tok_emb16", [V, C], F16, False)
    pos2_d = dp("pos2T", [C, TCH], F32, False)
    mask_d = dp("maskT", [P, 2, T], F16, False)
    wq_d = dp("wq", [L, C, C], F16, False)
    wk_d = dp("wk", [L, C, C], F16, False)
    wv_d = dp("wv", [L, C, C], F16, False)
    wo_d = dp("wo", [L, C, C], F16, False)
    w1_d = dp("w1", [L, C, FF], F16, False)
    w2_d = dp("w2", [L, FF, C], F16, False)
    bq_d = dp("bq", [L, C], F32, False)
    bk_d = dp("bk", [L, C], F32, False)
    bv_d = dp("bv", [L, C], F32, False)
    bo_d = dp("bo", [L, C], F32, False)
    b1_d = dp("b1", [L, FF], F32, False)
    b2_d = dp("b2", [L, C], F32, False)
    e8_d = dp("e8sel", [NCC, H, P], F32, False)
    wlm_d = dp("wlm", [C, V], F32, False)
    blm_d = dp("blm", [V], F32, False)
    out_d = dp("out", [NT, V], F32, True)

    with tile.TileContext(nc) as tc, ExitStack() as ctx:
        # ---------------- pools ----------------
        pconst = ctx.enter_context(tc.tile_pool(name="const", bufs=1))
        px = ctx.enter_context(tc.tile_pool(name="x", bufs=1))
        pw = ctx.enter_context(tc.tile_pool(name="w", bufs=1))
        pbias = ctx.enter_context(tc.tile_pool(name="bias", bufs=1))
        ph = ctx.enter_context(tc.tile_pool(name="h", bufs=2))
        pq = ctx.enter_context(tc.tile_pool(name="q", bufs=1))
        pv = ctx.enter_context(tc.tile_pool(name="v", bufs=2))
        po = ctx.enter_context(tc.tile_pool(name="o", bufs=1))
        pffn = ctx.enter_context(tc.tile_pool(name="ffn", bufs=1))
        psq = ctx.enter_context(tc.tile_pool(name="sq", bufs=2))
        pstat = ctx.enter_context(tc.tile_pool(name="stat", bufs=2))
        pe_ = ctx.enter_context(tc.tile_pool(name="e", bufs=6))
        prd = ctx.enter_context(tc.tile_pool(name="rd", bufs=4))
        plog = ctx.enter_context(tc.tile_pool(name="log", bufs=2))
        # PSUM pools (8 banks: mm 4 + scps 2 + ops 2; LN stats share scps)
        pmm = ctx.enter_context(tc.tile_pool(name="mm", bufs=4, space="PSUM"))
        psc = ctx.enter_context(tc.tile_pool(name="scps", bufs=2, space="PSUM"))
        pops = ctx.enter_context(tc.tile_pool(name="ops", bufs=2, space="PSUM"))

        # ---------------- constants ----------------
        ones_f = pconst.tile([P, 1], F32, tag="ones_f", name="ones_f")
        nc.vector.memset(ones_f, 1.0)
        ones = pconst.tile([P, 1], F32R, tag="ones", name="ones")
        nc.vector.tensor_copy(ones, ones_f)
        ones1_f = pconst.tile([1, P], F32, tag="ones1_f", name="ones1_f")
        nc.vector.memset(ones1_f, 1.0)
        ones1 = pconst.tile([1, P], F32R, tag="ones1", name="ones1")
        nc.vector.tensor_copy(ones1, ones1_f)
        eps_t = pconst.tile([1, 1], F32, tag="eps", name="eps")
        nc.vector.memset(eps_t, EPS)
        mask_sb = pconst.tile([P, 2, T], F16, tag="mask", name="mask")
        nc.sync.dma_start(out=mask_sb, in_=mask_d[:, :, :])
        tok_sb = pconst.tile([V, C], F16, tag="tok", name="tok")
        nc.sync.dma_start(out=tok_sb, in_=tok_d[:, :])
        wlm_sb = []
        for cc in range(NCC):
            t = pconst.tile([P, V], F32, tag=f"wlm{cc}", name=f"wlm{cc}")
            nc.sync.dma_start(out=t, in_=wlm_d[cc * P:(cc + 1) * P, :])
            wlm_sb.append(t)
        blm_bc = pconst.tile([P, V], F32, tag="blm", name="blm")
        nc.sync.dma_start(out=blm_bc, in_=_bcast_dram(blm_d[:], P))
        e8 = []
        for hq in range(NCC):
            f = pconst.tile([H, P], F32, tag=f"e8f{hq}", name=f"e8f{hq}")
            nc.sync.dma_start(out=f, in_=e8_d[hq])
            r8 = pconst.tile([H, P], F32R, tag=f"e8{hq}", name=f"e8{hq}")
            nc.vector.tensor_copy(r8, f)
            e8.append(r8)
        # one-hot columns for denominator matmuls: o8c[p, hh, j] = (j == hh)
        o8c = pconst.tile([P, H, H], F16, tag="o8c", name="o8c")
        nc.vector.memset(o8c, 0.0)
        for hh in range(H):
            nc.vector.memset(o8c[:, hh, hh:hh + 1], 1.0)

        # resident residual stream, feature-major: x_T[c, t]
        x_sb = [px.tile([P, NT], F32R, tag=f"x{cc}", name=f"x{cc}")
                for cc in range(NCC)]

        # ---------------- embedding ----------------
        with tc.tile_pool(name="emb", bufs=1) as pemb:
            oh_sb = pemb.tile([V, NT], F16, tag="oh", name="oh")
            nc.sync.dma_start(out=oh_sb, in_=onehot_d[:, :])
            pos_sb = []
            for cc in range(NCC):
                t = pemb.tile([P, TCH], F32, tag=f"pos{cc}", name=f"pos{cc}")
                nc.sync.dma_start(out=t, in_=pos2_d[cc * P:(cc + 1) * P, :])
                pos_sb.append(t)
            for ti in range(NTC):
                tsl = slice(ti * TCH, (ti + 1) * TCH)
                for cc in range(NCC):
                    ps = pmm.tile([P, TCH], F32, tag="mm", name="mmps")
                    nc.tensor.matmul(ps, tok_sb[:, cc * P:(cc + 1) * P],
                                     oh_sb[:, tsl], start=True, stop=True)
                    nc.vector.tensor_add(x_sb[cc][:, tsl], ps, pos_sb[cc])

        # ---------------- LN building blocks ----------------
        def ln_stats(tsl):
            """PE ones-matmul stats; returns psum tiles (sum, sumsq)."""
            S0 = psc.tile([1, TCH], F32, tag="scps", name="S0")
            S1 = psc.tile([1, TCH], F32, tag="scps", name="S1")
            for cc in range(NCC):
                sq = psq.tile([P, TCH], F32R, tag="sq", name="sq")
                nc.vector.tensor_mul(sq, x_sb[cc][:, tsl], x_sb[cc][:, tsl])
                nc.tensor.matmul(S0[0:1, :], _r(ones[:, :]), x_sb[cc][:, tsl],
                                 start=(cc == 0), stop=(cc == NCC - 1))
                nc.tensor.matmul(S1[0:1, :], _r(ones[:, :]), sq[:, :],
                                 start=(cc == 0), stop=(cc == NCC - 1))
            return S0, S1

        def ln_finish(S0, S1):
            """DVE finishing: returns (A=rstd, B=-mean*rstd) fp32r rows."""
            m_t = pstat.tile([1, TCH], F32R, tag="m", name="m_t")
            v_t = pstat.tile([1, TCH], F32R, tag="v", name="v_t")
            m2_t = pstat.tile([1, TCH], F32, tag="m2", name="m2_t")
            nc.vector.tensor_scalar_mul(m_t, S0[0:1, :], 1.0 / C)
            nc.vector.tensor_scalar_mul(v_t, S1[0:1, :], 1.0 / C)
            nc.vector.tensor_mul(m2_t, m_t, m_t)
            nc.vector.tensor_sub(v_t, v_t, m2_t)
            nc.scalar.activation(v_t, v_t, AF.Sqrt, bias=eps_t[:, :], scale=1.0)
            with nc.allow_low_precision("fp32r rstd is fp32-equivalent"):
                nc.vector.reciprocal(v_t, v_t)
            nc.vector.scalar_tensor_tensor(m_t, m_t, -1.0, v_t,
                                           op0=MULT, op1=MULT)
            return v_t, m_t

        def ln_bcast(v_t, m_t):
            a_ps = pmm.tile([P, TCH], F32, tag="mm", name="a_ps")
            nc.tensor.matmul(a_ps, _r(ones1[:, :]), v_t[:, :],
                             start=True, stop=True)
            b_ps = pmm.tile([P, TCH], F32, tag="mm", name="b_ps")
            nc.tensor.matmul(b_ps, _r(ones1[:, :]), m_t[:, :],
                             start=True, stop=True)
            return a_ps, b_ps

        def ln_apply(tsl, a_ps, b_ps, htag):
            h = []
            for cc in range(NCC):
                d = ph.tile([P, TCH], F16, tag=f"{htag}{cc}", name=f"h{cc}")
                nc.vector.tensor_mul(d, x_sb[cc][:, tsl], a_ps)
                nc.vector.tensor_add(d, d, b_ps)
                h.append(d)
            return h

        # ---------------- per-layer weights ----------------
        def load_weights(l):
            def _load(dram, tag, n, width):
                ts_ = []
                for i in range(n):
                    t = pw.tile([P, width], F16, tag=f"{tag}{i}", name=f"{tag}{i}")
                    nc.sync.dma_start(out=t, in_=dram[l, i * P:(i + 1) * P, :])
                    ts_.append(t)
                return ts_

            w = {}
            w["wq"] = _load(wq_d, "wq", NCC, C)
            w["wk"] = _load(wk_d, "wk", NCC, C)
            w["wv"] = _load(wv_d, "wv", NCC, C)
            w["wo"] = _load(wo_d, "wo", NCC, C)
            w["w1"] = _load(w1_d, "w1", NCC, FF)
            w["w2"] = _load(w2_d, "w2", NFF, C)
            if not zero_attn_bias:
                for nm, dr in (("bq", bq_d), ("bk", bk_d), ("bo", bo_d)):
                    t = pbias.tile([P, NCC], F32, tag=nm, name=nm)
                    nc.sync.dma_start(out=t, in_=dr[l].rearrange("(a p) -> p a", p=P))
                    w[nm] = t
                bv_bc = pbias.tile([P, C], F32, tag="bvb", name="bvb")
                nc.sync.dma_start(out=bv_bc, in_=_bcast_dram(bv_d[l], P))
                w["bv_bc"] = bv_bc
            if not zero_mlp_bias:
                t = pbias.tile([P, NFF], F32, tag="b1", name="b1")
                nc.sync.dma_start(out=t, in_=b1_d[l].rearrange("(a p) -> p a", p=P))
                w["b1"] = t
                t = pbias.tile([P, NCC], F32, tag="b2", name="b2")
                nc.sync.dma_start(out=t, in_=b2_d[l].rearrange("(a p) -> p a", p=P))
                w["b2"] = t
            return w

        # pending FFN2 emission (software pipelining across tc steps)
        def ffn2_emit(st, cc_list):
            w2_sb, ffn1, ptsl, wt_ = st
            for cc in cc_list:
                ps = pmm.tile([P, TCH], F32, tag="mm", name="mmps")
                for fc in range(NFF):
                    nc.tensor.matmul(ps, w2_sb[fc][:, cc * P:(cc + 1) * P],
                                     ffn1[fc][:, :], start=(fc == 0),
                                     stop=(fc == NFF - 1))
                if zero_mlp_bias:
                    nc.vector.tensor_add(x_sb[cc][:, ptsl], ps,
                                         x_sb[cc][:, ptsl])
                else:
                    nc.vector.scalar_tensor_tensor(
                        x_sb[cc][:, ptsl], ps, wt_["b2"][:, cc:cc + 1],
                        x_sb[cc][:, ptsl], op0=ADD, op1=ADD)

        pending = None  # (w2_sb, ffn1, tsl, wt)

        # ---------------- transformer layers ----------------
        for l in range(L):
            wt = load_weights(l)
            for ti in range(NTC):
                tsl = slice(ti * TCH, (ti + 1) * TCH)

                # LN1, interleaved with previous chunk's FFN2 on PE
                S0, S1 = ln_stats(tsl)
                AB = ln_finish(S0, S1)
                if pending is not None:
                    ffn2_emit(pending, [0, 1])
                a_ps, b_ps = ln_bcast(*AB)
                if pending is not None:
                    ffn2_emit(pending, [2, 3])
                    pending = None
                h1 = ln_apply(tsl, a_ps, b_ps, "h")

                # ---- QKV ----
                q_t, k_t = [], []
                for dst, wsb, bnm in ((q_t, wt["wq"], "bq"),
                                      (k_t, wt["wk"], "bk")):
                    for hq in range(NCC):
                        ps = pmm.tile([P, TCH], F32, tag="mm", name="mmps")
                        for cc in range(NCC):
                            nc.tensor.matmul(ps, wsb[cc][:, hq * P:(hq + 1) * P],
                                             h1[cc][:, :], start=(cc == 0),
                                             stop=(cc == NCC - 1))
                        qt = pq.tile([P, TCH], F16, tag=f"{bnm}t{hq}",
                                     name=f"{bnm}t{hq}")
                        if zero_attn_bias:
                            nc.scalar.copy(qt, ps)
                        else:
                            nc.scalar.activation(qt, ps, AF.Identity,
                                                 bias=wt[bnm][:, hq:hq + 1],
                                                 scale=1.0)
                        dst.append(qt)
                v8 = []
                for tt in range(TCH // P):
                    ps = pmm.tile([P, C], F32, tag="mm", name="mmps")
                    for cc in range(NCC):
                        nc.tensor.matmul(ps, h1[cc][:, tt * P:(tt + 1) * P],
                                         wt["wv"][cc][:, :], start=(cc == 0),
                                         stop=(cc == NCC - 1))
                    vt = pv.tile([P, H, HD], F16, tag=f"v{tt}", name=f"vt{tt}")
                    if zero_attn_bias:
                        nc.scalar.copy(vt, ps[:].rearrange("p (h d) -> p h d", h=H))
                    else:
                        nc.vector.tensor_add(
                            vt, ps[:].rearrange("p (h d) -> p h d", h=H),
                            wt["bv_bc"][:].rearrange("p (h d) -> p h d", h=H))
                    v8.append(vt)

                # ---- attention (per batch item x head, pipelined) ----
                # causal structure: the s-chunk-1 x t<128 quadrant is fully
                # masked, so its scores/exp/o contributions are skipped.
                o_t = [po.tile([P, TCH], F16, tag=f"o{hq}", name=f"ot{hq}")
                       for hq in range(NCC)]
                for bi in range(BI):
                    den_ps = pmm.tile([H, T], F32, tag="mm", name="den_ps")
                    for hh in range(H):
                        hq, hr = divmod(hh, 2)
                        rsl = slice(hr * HD, (hr + 1) * HD)
                        qsl = q_t[hq][rsl, bi * T:(bi + 1) * T]
                        ksl0 = k_t[hq][rsl, bi * T: bi * T + P]
                        ksl1 = k_t[hq][rsl, bi * T + P: bi * T + 2 * P]
                        sc_ps = psc.tile([P, 2, T], F32, tag="scps", name="scps")
                        nc.tensor.matmul(sc_ps[:, 0, :], ksl0, qsl,
                                         start=True, stop=True)
                        nc.tensor.matmul(sc_ps[:, 1, P:T], ksl1, qsl[:, P:T],
                                         start=True, stop=True)
                        e = pe_.tile([P, 2, T], F16, tag="e", name="e")
                        nc.scalar.activation(e[:, 0, :], sc_ps[:, 0, :], AF.Exp)
                        nc.scalar.activation(e[:, 1, P:T], sc_ps[:, 1, P:T],
                                             AF.Exp)
                        nc.gpsimd.tensor_mul(e[:, 0, :], e[:, 0, :],
                                             mask_sb[:, 0, :])
                        nc.gpsimd.tensor_mul(e[:, 1, P:T], e[:, 1, P:T],
                                             mask_sb[:, 1, P:T])
                        o_ps = pops.tile([HD, T], F32, tag="ops", name="ops")
                        nc.tensor.matmul(o_ps, v8[bi * 2][:, hh, :], e[:, 0, :],
                                         start=True, stop=False)
                        nc.tensor.matmul(o_ps[:, P:T], v8[bi * 2 + 1][:, hh, :],
                                         e[:, 1, P:T], start=False, stop=True)
                        nc.tensor.matmul(den_ps, o8c[:, hh, :], e[:, 0, :],
                                         start=(hh == 0), stop=False)
                        nc.tensor.matmul(den_ps[:, P:T], o8c[:, hh, :],
                                         e[:, 1, P:T], start=False,
                                         stop=(hh == H - 1))
                        nc.scalar.copy(o_t[hq][rsl, bi * T:(bi + 1) * T],
                                       o_ps[0:HD, :])
                    rden = prd.tile([H, T], F32R, tag="rden", name="rden")
                    with nc.allow_low_precision("fp32r rden is fp32-equivalent"):
                        nc.vector.reciprocal(rden, den_ps)
                    for hq in range(NCC):
                        rdb = pmm.tile([P, T], F32, tag="mm", name="rdb")
                        nc.tensor.matmul(rdb, e8[hq][:, :], rden[:, :],
                                         start=True, stop=True)
                        osl = o_t[hq][:, bi * T:(bi + 1) * T]
                        nc.vector.tensor_mul(osl, osl, rdb)

                # ---- proj + residual ----
                for cc in range(NCC):
                    ps = pmm.tile([P, TCH], F32, tag="mm", name="mmps")
                    for hq in range(NCC):
                        nc.tensor.matmul(ps, wt["wo"][hq][:, cc * P:(cc + 1) * P],
                                         o_t[hq][:, :], start=(hq == 0),
                                         stop=(hq == NCC - 1))
                    if zero_attn_bias:
                        nc.vector.tensor_add(x_sb[cc][:, tsl], ps,
                                             x_sb[cc][:, tsl])
                    else:
                        nc.vector.scalar_tensor_tensor(
                            x_sb[cc][:, tsl], ps, wt["bo"][:, cc:cc + 1],
                            x_sb[cc][:, tsl], op0=ADD, op1=ADD)

                # ---- LN2 + FFN1 ----
                S0, S1 = ln_stats(tsl)
                AB = ln_finish(S0, S1)
                a_ps, b_ps = ln_bcast(*AB)
                h2 = ln_apply(tsl, a_ps, b_ps, "g")
                ffn1 = []
                for fc in range(NFF):
                    ps = pmm.tile([P, TCH], F32, tag="mm", name="mmps")
                    for cc in range(NCC):
                        nc.tensor.matmul(ps, wt["w1"][cc][:, fc * P:(fc + 1) * P],
                                         h2[cc][:, :], start=(cc == 0),
                                         stop=(cc == NCC - 1))
                    ft = pffn.tile([P, TCH], F16, tag=f"f{fc}", name=f"ft{fc}")
                    if zero_mlp_bias:
                        nc.scalar.activation(ft, ps, AF.Relu)
                    else:
                        nc.scalar.activation(ft, ps, AF.Relu,
                                             bias=wt["b1"][:, fc:fc + 1],
                                             scale=1.0)
                    ffn1.append(ft)
                pending = (wt["w2"], ffn1, tsl, wt)

        ffn2_emit(pending, [0, 1, 2, 3])
        pending = None

        # ---------------- lm head ----------------
        for tt in range(NT // P):
            ps = pmm.tile([P, V], F32, tag="mm", name="mmps")
            for cc in range(NCC):
                nc.tensor.matmul(ps, x_sb[cc][:, tt * P:(tt + 1) * P].bitcast(F32),
                                 wlm_sb[cc][:, :], start=(cc == 0),
                                 stop=(cc == NCC - 1))
            lo = plog.tile([P, V], F32, tag="lg", name="lo")
            nc.vector.tensor_add(lo, ps, blm_bc)
            nc.sync.dma_start(out=out_d[tt * P:(tt + 1) * P, :], in_=lo)

    if not nc.is_finalized():
        nc.finalize()
    return nc


def prep_inputs(idx, tok_emb, pos_emb, Wq, Wk, Wv, Wo, bo, ln1_g, ln1_b,
                ln2_g, ln2_b, W1, b1, W2, b2, Wlm, blm):
    """host-side: fold LN affines into weights, build per-core input maps"""
    f32 = np.float32
    idx = np.asarray(idx)
    tok_emb = np.asarray(tok_emb, f32)
    pos_emb = np.asarray(pos_emb, f32)
    scale = C ** -0.5

    wq = np.empty((L, C, C), f32)
    wk = np.empty((L, C, C), f32)
    wv = np.empty((L, C, C), f32)
    wo = np.empty((L, C, C), f32)
    w1 = np.empty((L, C, FF), f32)
    w2 = np.empty((L, FF, C), f32)
    bq = np.empty((L, C), f32)
    bk = np.empty((L, C), f32)
    bv = np.empty((L, C), f32)
    b1f = np.empty((L, FF), f32)
    for l in range(L):
        wq_c = np.asarray(Wq[l], f32).transpose(1, 0, 2).reshape(C, C)
        wk_c = np.asarray(Wk[l], f32).transpose(1, 0, 2).reshape(C, C)
        wv_c = np.asarray(Wv[l], f32).transpose(1, 0, 2).reshape(C, C)
        g1 = np.asarray(ln1_g[l], f32)[:, None]
        b1_ = np.asarray(ln1_b[l], f32)
        g2 = np.asarray(ln2_g[l], f32)[:, None]
        b2_ = np.asarray(ln2_b[l], f32)
        wq[l] = g1 * wq_c * scale
        bq[l] = (b1_ @ wq_c) * scale
        wk[l] = g1 * wk_c
        bk[l] = b1_ @ wk_c
        wv[l] = g1 * wv_c
        bv[l] = b1_ @ wv_c
        wo[l] = np.asarray(Wo[l], f32)
        w1[l] = g2 * np.asarray(W1[l], f32)
        b1f[l] = np.asarray(b1[l], f32) + b2_ @ np.asarray(W1[l], f32)
        w2[l] = np.asarray(W2[l], f32)

    bo = np.asarray(bo, f32)
    b2a = np.asarray(b2, f32)

    # causal mask in [s%128, s//128, t] layout
    s_g = np.arange(2 * P).reshape(2, P).T          # [128, 2] global s
    mask = (s_g[:, :, None] <= np.arange(T)[None, None, :]).astype(np.float16)

    pos2 = np.concatenate([pos_emb.T, pos_emb.T], axis=1)  # [C, 512]

    flags = {
        "zero_attn_bias": not (np.any(bq) or np.any(bk) or np.any(bv)
                               or np.any(bo)),
        "zero_mlp_bias": not (np.any(b1f) or np.any(b2a)),
    }

    e8sel = np.zeros((NCC, H, P), f32)
    for hq in range(NCC):
        for p_ in range(P):
            e8sel[hq, 2 * hq + p_ // HD, p_] = 1.0

    shared = {
        "e8sel": e8sel,
        "tok_emb16": tok_emb.astype(np.float16),
        "pos2T": np.ascontiguousarray(pos2, f32),
        "maskT": np.ascontiguousarray(mask),
        "wq": wq.astype(np.float16), "wk": wk.astype(np.float16),
        "wv": wv.astype(np.float16), "wo": wo.astype(np.float16),
        "w1": w1.astype(np.float16), "w2": w2.astype(np.float16),
        "bq": bq, "bk": bk, "bv": bv,
        "bo": bo, "b1": b1f, "b2": b2a,
        "wlm": np.asarray(Wlm, f32), "blm": np.asarray(blm, f32),
    }
    in_maps = []
    vocab = np.arange(V)
    for core in range(NCORES):
        toks = np.asarray(idx[core * BL:(core + 1) * BL]).reshape(-1)
        oh = (vocab[:, None] == toks[None, :]).astype(np.float16)
        m = dict(shared)
        m["onehotT"] = np.ascontiguousarray(oh)
        in_maps.append(m)
    return in_maps, flags


_NC_CACHE = {}


def get_nc(flags=None):
    if flags is None:
        flags = {"zero_attn_bias": False, "zero_mlp_bias": False}
    key = (flags["zero_attn_bias"], flags["zero_mlp_bias"])
    if key not in _NC_CACHE:
        _NC_CACHE[key] = build_bass(**flags)
    return _NC_CACHE[key]


def run(in_maps, flags=None, trace=False, **kw):
    from concourse.bass_utils import run_bass_kernel_spmd
    nc = get_nc(flags)
    return run_bass_kernel_spmd(nc, in_maps, list(range(NCORES)),
                                trace=trace, **kw)


def kernel(**inputs):
    in_maps, flags = prep_inputs(**inputs)
    res = run(in_maps, flags)
    outs = [res.results[i]["out"].reshape(BL, T, V) for i in range(NCORES)]
    return np.concatenate(outs, axis=0).astype(np.float32)
